# revision 14
# baseline (speedup 1.0000x reference)
"""Trainium2 Bass kernel for nn_AttentionProbe_80891414053184.

Math (reference):
    y  = relu(x @ W1.T + b1)            # (B,S,H) -> (B,S,128)
    y2 = relu(y @ W2.T + b2)            # (B,S,128)
    l  = y2 @ Wq.T + pos*pos_w  (+mask) # (B,S,8) logits
    p  = softmax(l, axis=S)
    v  = y2 @ Wv.T + bv
    out[b] = sum_{s,h} p*v + bias       # (B,1)

Strategy: sequence-parallel over 8 cores (512 positions x 4 batches = 2048
tokens per core).  Each core streams its x-shard, runs the MLP + head
projections on-chip, and emits per-(batch, head) partial softmax stats
(Z=sum exp(l-K), W'=sum exp(l-K)*v_raw) for a HOST-CHOSEN shift K (the
max of the ALiBi ramp + mask term over the shard, known without looking
at the data).  The host merges the 8 partial stats with the standard
online-softmax combine (m=K per core) and produces the (4,1) output.

Perf decisions (from HW traces of earlier versions):
 - x and W1 travel as fp8 e4m3 (W1 pre-scaled by 64 so its sigma~1/64
   values leave the fp8 denormal range; 1/64 is folded into W2, exact by
   relu's positive homogeneity).  Halves the dominant HBM stream vs bf16.
 - Layer-1 matmuls run perf_mode=DoubleRow: K=256 per instruction.
 - x streams via BOTH HWDGE rings (nc.sync + nc.scalar) in 1MB transfers,
   soft-dep-chained so per-ring delivery order == PE consumption order
   (the tile scheduler otherwise reorders and starves the PE mid-stream).
 - W2/Wq/Wv and the MLP tail activations run in bf16 (halves the const
   stream and doubles DVE throughput); measured end-to-end rel-err ~6e-3
   against a 2e-2 gate.
 - The whole fp8 x-shard (64KB/partition) stays resident in SBUF.
 - The last super-chunk pair is DMA'd per token tile so tile t's MLP tail
   overlaps tile t+1's final transfer.
 - ~14 dummy matmuls on zeroed SBUF warm the PE HAM clock gate during the
   first DMA wait.
 - No on-device softmax max-reduce and no bv add: K is baked into the
   additive term `ca`, and W = W' + bv*Z happens in the host merge.
"""

import numpy as np

# Problem dims (hardcoded per harness contract).
B, S, H = 4, 4096, 4096
MLP, NH = 128, 8
NCORES = 8
S_SHARD = S // NCORES        # 512 seq positions per core
TOK = B * S_SHARD            # 2048 tokens per core
NT = TOK // 512              # 4 token tiles of 512 (= one batch each)
KCH = H // 128               # 32 contraction chunks of 128
NSUP = KCH // 2              # 16 DoubleRow super-chunks of 256
NBIG = (NSUP - 2) // 2       # 7 big 1MB x transfers (super-chunks 0..13)
P32 = NT * NH                # 32 packed (tile, head) lanes
W1_SCALE = 64.0              # lifts W1 (sigma 1/64) out of fp8 denormals

_cache = {}


def _build_nc():
    import concourse.mybir as mybir
    import concourse.tile as tile
    from concourse import bacc
    from concourse.tile import add_dep_helper

    f32 = mybir.dt.float32
    bf16 = mybir.dt.bfloat16
    fp8 = mybir.dt.float8e4
    DR = mybir.MatmulPerfMode.DoubleRow

    AF = mybir.ActivationFunctionType
    AX = mybir.AxisListType
    OP = mybir.AluOpType
    CQ = MLP                    # wq32 blocks start (cols of cwr)
    CV = MLP + P32 * NT         # wv32 blocks start

    nc = bacc.Bacc()
    # x, packed on host: xt[p, k, n] = x^T[k*128+p, n]  (fp8)
    xt_d = nc.dram_tensor("xt", [128, NSUP, 2, TOK], fp8, kind="ExternalInput")
    # W1*64 packed likewise: w1s[p, k, m] = 64*W1[m, k*128+p]  (fp8)
    w1_d = nc.dram_tensor("w1s", [128, NSUP, 2, MLP], fp8, kind="ExternalInput")
    # cwr: [W2.T/64 | wq32 (4 x 32-wide zero-padded blocks) | wv32]  (bf16)
    cwr_d = nc.dram_tensor("cwr", [MLP, MLP + 2 * P32 * NT], bf16,
                           kind="ExternalInput")
    # cb: bias columns [64*b1 | b2]  (f32)
    cb_d = nc.dram_tensor("cb", [MLP, 2], f32, kind="ExternalInput")
    ca_d = nc.dram_tensor("ca", [P32, 512], f32, kind="ExternalInput")
    st_d = nc.dram_tensor("stats", [P32, 2], f32, kind="ExternalOutput")

    with tile.TileContext(nc) as tc:
        with (
            tc.tile_pool(name="const", bufs=1) as const,
            tc.tile_pool(name="xp", bufs=(NSUP - 2) + NT) as xp,
            tc.tile_pool(name="yp", bufs=4) as yp,
            tc.tile_pool(name="y2p", bufs=4) as y2p,
            tc.tile_pool(name="smallp", bufs=1) as smallp,
            tc.tile_pool(name="statsp", bufs=1) as statsp,
            tc.tile_pool(name="ps_y", bufs=4, space="PSUM") as ps_y,
            tc.tile_pool(name="ps_y2", bufs=2, space="PSUM") as ps_y2,
            tc.tile_pool(name="ps_q", bufs=1, space="PSUM") as ps_q,
            tc.tile_pool(name="ps_v", bufs=1, space="PSUM") as ps_v,
        ):
            # --- HAM warmup: zeroed fp8 tiles + dummy matmuls keep the PE
            # busy through one full 4096-cycle activity window during the
            # first x DMA, so the clock gate opens before real work lands.
            warm_w = const.tile([128, 2, MLP], fp8)
            nc.gpsimd.memset(warm_w[:], 0.0)
            warm_x = const.tile([128, 2, 512], fp8)
            nc.gpsimd.memset(warm_x[:], 0.0)
            warm_ps = ps_y2.tile([128, 512], f32, tag="y2", name="warm_ps")
            warm_mm = None
            for i in range(18):
                warm_mm = nc.tensor.matmul(warm_ps[:], warm_w[:], warm_x[:],
                                           start=True, stop=True,
                                           perf_mode=DR)

            # --- DMA plan.  Ring A (sync): x0,x2,x4,x6,xl0,xl2,stats.
            # Ring B (scalar): w1,x1,x3,x5,xl1,xl3.  Small consts on the
            # SWDGE ring.  Soft deps pin per-ring issue order; the lane
            # sems then deliver in consumption order.
            last_on = {}

            def ring_dma(ring_key, ring, **kw):
                dma = ring.dma_start(**kw)
                prev = last_on.get(ring_key)
                if prev is not None:
                    add_dep_helper(dma.ins, prev.ins, sync=False,
                                   reason="ring issue order")
                last_on[ring_key] = dma
                return dma

            w1_sb = const.tile([128, NSUP, 2, MLP], fp8)
            ring_dma('B', nc.scalar, out=w1_sb[:], in_=w1_d[:])
            ca_sb = const.tile([P32, 512], f32)
            nc.gpsimd.dma_start(out=ca_sb[:], in_=ca_d[:])
            cb_sb = const.tile([MLP, 2], f32)
            nc.gpsimd.dma_start(out=cb_sb[:], in_=cb_d[:])
            cwr_sb = const.tile([MLP, MLP + 2 * P32 * NT], bf16)
            nc.gpsimd.dma_start(out=cwr_sb[:], in_=cwr_d[:])

            stats_sb = statsp.tile([P32, 2], f32)

            # One 512KB transfer per super-chunk, strictly alternating
            # rings: fine granularity keeps chunk latency ~1.5us so the PE
            # never idles past the HAM re-throttle window during the ramp.
            rings = [('A', nc.sync), ('B', nc.scalar)]
            x_tiles = []
            for g in range(NSUP - 2):
                x_sb = xp.tile([128, 2, TOK], fp8, tag="x", name=f"x{g}")
                rk, ring = rings[g % 2]
                ring_dma(rk, ring, out=x_sb[:], in_=xt_d[:, g])
                x_tiles.append(x_sb)
            xl_tiles = []
            for t in range(NT):
                xl_sb = xp.tile([128, 2, 2, 512], fp8, tag="xl", name=f"xl{t}")
                rk, ring = rings[t % 2]
                ring_dma(rk, ring, out=xl_sb[:],
                         in_=xt_d[:, NSUP - 2:NSUP, :, t * 512:(t + 1) * 512])
                xl_tiles.append(xl_sb)

            # --- Layer 1: yT[t] (128, 512) += (64*W1)^T-chunk @ x-chunk,
            # DoubleRow accumulation over 16 super-chunks of K=256.
            psum_y = [ps_y.tile([128, 512], f32, tag="y", name=f"y_ps{t}")
                      for t in range(NT)]
            for g in range(NSUP - 2):
                xg = x_tiles[g]
                for t in range(NT):
                    mm = nc.tensor.matmul(
                        psum_y[t][:],
                        w1_sb[:, g],
                        xg[:, :, t * 512:(t + 1) * 512],
                        start=(g == 0), stop=False,
                        perf_mode=DR,
                    )
                    if g == 0 and t == 0 and warm_mm is not None:
                        add_dep_helper(mm.ins, warm_mm.ins, sync=False,
                                       reason="warmups before first mm")
            for t in range(NT):
                nc.tensor.matmul(psum_y[t][:], w1_sb[:, NSUP - 2],
                                 xl_tiles[t][:, 0],
                                 start=False, stop=False, perf_mode=DR)
                nc.tensor.matmul(psum_y[t][:], w1_sb[:, NSUP - 1],
                                 xl_tiles[t][:, 1],
                                 start=False, stop=True, perf_mode=DR)

            # cb/ca lane warmups (each engine observes the const lanes once
            # so steady-state instructions carry at most one new wait).
            warm_act = const.tile([MLP, 1], f32)
            nc.scalar.copy(out=warm_act[:], in_=cb_sb[:, 0:1])
            warm_dve = const.tile([P32, 1], f32)
            nc.vector.tensor_copy(out=warm_dve[:], in_=ca_sb[:, 0:1])
            warm_pe2 = ps_y2.tile([128, 512], f32, tag="y2", name="warm_pe2")
            nc.tensor.matmul(warm_pe2[0:NH, 0:NH], cwr_sb[:, 0:NH],
                             cwr_sb[:, 0:NH], start=True, stop=True)

            # --- MLP tail + head projections per token tile (bf16).
            q32_ps = ps_q.tile([P32, 512], f32, tag="q", name="q32_ps")
            v32_ps = ps_v.tile([P32, 512], f32, tag="v", name="v32_ps")
            for t in range(NT):
                y_sb = yp.tile([128, 512], bf16, tag="ysb", name=f"y_sb{t}")
                # relu on DVE (add+max) keeps ACT free for relu2/exp; the
                # 64x scale rides along and is cancelled by W2/64 in cwr.
                nc.vector.tensor_scalar(out=y_sb[:], in0=psum_y[t][:],
                                        scalar1=cb_sb[:, 0:1],
                                        scalar2=0.0, op0=OP.add, op1=OP.max)
                y2_ps = ps_y2.tile([128, 512], f32, tag="y2", name=f"y2_ps{t}")
                nc.tensor.matmul(y2_ps[:], cwr_sb[:, 0:MLP], y_sb[:],
                                 start=True, stop=True)
                y2_sb = y2p.tile([128, 512], bf16, tag="y2sb", name=f"y2_sb{t}")
                nc.scalar.activation(out=y2_sb[:], in_=y2_ps[:], func=AF.Relu,
                                     bias=cb_sb[:, 1:2], scale=1.0)
                # Head projections: the (128, 32) weight block for tile t is
                # zero outside rows 8t..8t+8, so accumulating all 4 tiles into
                # one (32, 512) bank packs q/v as (tile, head) x seq lanes.
                nc.tensor.matmul(q32_ps[:],
                                 cwr_sb[:, CQ + P32 * t:CQ + P32 * (t + 1)],
                                 y2_sb[:], start=(t == 0), stop=(t == NT - 1))
                nc.tensor.matmul(v32_ps[:],
                                 cwr_sb[:, CV + P32 * t:CV + P32 * (t + 1)],
                                 y2_sb[:], start=(t == 0), stop=(t == NT - 1))

            # --- Softmax stats over the packed (32, 512) lanes.
            # ca already contains ramp + mask - K, so l' = q + ca is the
            # shifted logit; no max-reduce needed on device.
            l_sb = smallp.tile([P32, 512], f32, tag="l", name="l_sb")
            nc.vector.tensor_add(out=l_sb[:], in0=q32_ps[:],
                                 in1=ca_sb[:])
            e_sb = smallp.tile([P32, 512], bf16, tag="e", name="e_sb")
            # e = exp(l'); stats[:, 0] = Z = sum e (accumulated in f32)
            nc.scalar.activation(out=e_sb[:], in_=l_sb[:], func=AF.Exp,
                                 bias=0.0, scale=1.0,
                                 accum_out=stats_sb[:, 0:1])
            ev_sb = smallp.tile([P32, 512], bf16, tag="ev", name="ev_sb")
            nc.vector.tensor_mul(out=ev_sb[:], in0=e_sb[:], in1=v32_ps[:])
            # stats[:, 1] = W' = sum e*v_raw   (bv folded in on host)
            nc.vector.tensor_reduce(out=stats_sb[:, 1:2], in_=ev_sb[:],
                                    axis=AX.X, op=OP.add)

            ring_dma('A', nc.sync, out=st_d[:], in_=stats_sb[:])

    nc.finalize()
    return nc


def get_nc():
    if "nc" not in _cache:
        _cache["nc"] = _build_nc()
    return _cache["nc"]


def make_core_inputs(x, mask, W1, b1, W2, b2, Wq, Wv, bv, pos_w, bias):
    """Host-side shard + transpose + fp8 quantization.

    Returns (in_maps, K) where K[c, t, h] is the logit shift baked into
    core c's `ca` (the host-known max of ramp+mask over the shard)."""
    import ml_dtypes
    fp8 = ml_dtypes.float8_e4m3
    bf16 = ml_dtypes.bfloat16

    # w1s[p, k, m] = 64*W1[m, k*128+p], fp8
    w1s = np.ascontiguousarray(
        (W1 * W1_SCALE).reshape(MLP, KCH, 128).transpose(2, 1, 0)
    ).astype(fp8).reshape(128, NSUP, 2, MLP)

    cwr = np.zeros((MLP, MLP + 2 * P32 * NT), dtype=np.float32)
    cwr[:, 0:MLP] = W2.T / W1_SCALE
    # zero-padded per-tile head blocks: block t covers psum rows 8t..8t+8
    for t in range(NT):
        cwr[:, MLP + P32 * t + NH * t:MLP + P32 * t + NH * (t + 1)] = Wq.T
        base_v = MLP + P32 * NT
        cwr[:, base_v + P32 * t + NH * t:base_v + P32 * t + NH * (t + 1)] = \
            Wv.T
    cwr = cwr.astype(bf16)
    cb = np.stack([b1 * W1_SCALE, b2], axis=1).astype(np.float32)  # (MLP, 2)
    pos = np.arange(S, dtype=np.float32)
    maskadd = np.where(mask == 0, np.float32(-1e9), np.float32(0.0))  # (B,S)

    in_maps = []
    K_all = np.empty((NCORES, NT, NH), dtype=np.float64)
    for c in range(NCORES):
        sl = slice(c * S_SHARD, (c + 1) * S_SHARD)
        # xt[p, k, n] = x^T[k*128+p, n] for this shard's 2048 tokens
        xt = np.ascontiguousarray(
            x[:, sl, :].astype(fp8).transpose(2, 0, 1).reshape(H, TOK)
            .reshape(KCH, 128, TOK).transpose(1, 0, 2)
        ).reshape(128, NSUP, 2, TOK)
        add_ths = (pos_w.astype(np.float64)[None, :, None]
                   * pos[sl].astype(np.float64)[None, None, :]
                   + maskadd[:, None, sl])           # (B=NT, NH, 512)
        # K: host-known shift; floor keeps exp() sane when fully masked
        K = np.maximum(add_ths.max(axis=2), -60.0)   # (NT, NH)
        K_all[c] = K
        ca = (add_ths - K[:, :, None]).astype(np.float32).reshape(P32, 512)
        in_maps.append({"xt": xt, "w1s": w1s, "cwr": cwr, "cb": cb,
                        "ca": ca})
    return in_maps, K_all


def merge_stats(stats_all, K_all, bv, bias):
    """stats_all: (NCORES, 32, 2) = [Z, W'] per (batch tile, head) lane
    under shift K_all[c, t, h]; W = W' + bv*Z -> (B, 1) output."""
    st = np.asarray(stats_all, dtype=np.float64).reshape(NCORES, NT, NH, 2)
    m = K_all                # (C, B, NH): logits were shifted by -K
    Z = st[..., 0]
    W = st[..., 1] + bv.astype(np.float64)[None, None, :] * Z
    M = m.max(axis=0)        # (B, NH)
    alpha = np.exp(m - M[None])
    Zg = (alpha * Z).sum(axis=0)
    Wg = (alpha * W).sum(axis=0)
    out = (Wg / Zg).sum(axis=1)          # (B,)
    return (out[:, None] + np.float64(bias.reshape(1)[0])).astype(np.float32)


def kernel(x, mask, W1, b1, W2, b2, Wq, Wv, bv, pos_w, bias, _trace=False):
    from concourse.bass_utils import run_bass_kernel_spmd

    x = np.asarray(x, dtype=np.float32)
    args = [np.asarray(a) for a in (W1, b1, W2, b2, Wq, Wv, bv, pos_w, bias)]
    in_maps, K_all = make_core_inputs(x, np.asarray(mask), *args)
    nc = get_nc()
    res = run_bass_kernel_spmd(nc, in_maps, core_ids=list(range(NCORES)),
                               trace=_trace)
    stats_all = np.stack([r["stats"] for r in res.results])  # (C, 32, 2)
    out = merge_stats(stats_all, K_all, args[6], args[8])
    if _trace:
        kernel.last_result = res
    return out


# revision 20
# speedup vs baseline: 1.0010x; 1.0010x over previous
"""Trainium2 Bass kernel for nn_AttentionProbe_80891414053184.

Math (reference):
    y  = relu(x @ W1.T + b1)            # (B,S,H) -> (B,S,128)
    y2 = relu(y @ W2.T + b2)            # (B,S,128)
    l  = y2 @ Wq.T + pos*pos_w  (+mask) # (B,S,8) logits
    p  = softmax(l, axis=S)
    v  = y2 @ Wv.T + bv
    out[b] = sum_{s,h} p*v + bias       # (B,1)

Strategy: sequence-parallel over 8 cores (512 positions x 4 batches = 2048
tokens per core).  Each core streams its x-shard, runs the MLP + head
projections on-chip, and emits per-(batch, head) partial softmax stats
(Z=sum exp(l-K), W'=sum exp(l-K)*v_raw) for a HOST-CHOSEN shift K (the
max of the ALiBi ramp + mask term over the shard, known without looking
at the data).  The host merges the 8 partial stats with the standard
online-softmax combine (m=K per core) and produces the (4,1) output.

Perf decisions (from HW traces of earlier versions):
 - x and W1 travel as fp8 e4m3 (W1 pre-scaled by 64 so its sigma~1/64
   values leave the fp8 denormal range; 1/64 is folded into W2, exact by
   relu's positive homogeneity).  Halves the dominant HBM stream vs bf16.
 - Layer-1 matmuls run perf_mode=DoubleRow: K=256 per instruction.
 - x streams via BOTH HWDGE rings (nc.sync + nc.scalar) in 1MB transfers,
   soft-dep-chained so per-ring delivery order == PE consumption order
   (the tile scheduler otherwise reorders and starves the PE mid-stream).
 - W2/Wq/Wv and the MLP tail activations run in bf16 (halves the const
   stream and doubles DVE throughput); measured end-to-end rel-err ~6e-3
   against a 2e-2 gate.
 - The whole fp8 x-shard (64KB/partition) stays resident in SBUF.
 - The last super-chunk pair is DMA'd per token tile so tile t's MLP tail
   overlaps tile t+1's final transfer.
 - ~14 dummy matmuls on zeroed SBUF warm the PE HAM clock gate during the
   first DMA wait.
 - No on-device softmax max-reduce and no bv add: K is baked into the
   additive term `ca`, and W = W' + bv*Z happens in the host merge.
"""

import numpy as np

# Problem dims (hardcoded per harness contract).
B, S, H = 4, 4096, 4096
MLP, NH = 128, 8
NCORES = 8
S_SHARD = S // NCORES        # 512 seq positions per core
TOK = B * S_SHARD            # 2048 tokens per core
NT = TOK // 512              # 4 token tiles of 512 (= one batch each)
KCH = H // 128               # 32 contraction chunks of 128
NSUP = KCH // 2              # 16 DoubleRow super-chunks of 256
NBIG = (NSUP - 2) // 2       # 7 big 1MB x transfers (super-chunks 0..13)
P32 = NT * NH                # 32 packed (tile, head) lanes
W1_SCALE = 64.0              # lifts W1 (sigma 1/64) out of fp8 denormals

_cache = {}


def _build_nc():
    import concourse.mybir as mybir
    import concourse.tile as tile
    from concourse import bacc
    from concourse.tile import add_dep_helper

    f32 = mybir.dt.float32
    bf16 = mybir.dt.bfloat16
    fp8 = mybir.dt.float8e4
    DR = mybir.MatmulPerfMode.DoubleRow

    AF = mybir.ActivationFunctionType
    AX = mybir.AxisListType
    OP = mybir.AluOpType
    CQ = MLP                    # wq32 blocks start (cols of cwr)
    CV = MLP + P32 * NT         # wv32 blocks start

    nc = bacc.Bacc()
    # x, packed on host: xt[p, k, n] = x^T[k*128+p, n]  (fp8)
    xt_d = nc.dram_tensor("xt", [128, NSUP, 2, TOK], fp8, kind="ExternalInput")
    # W1*64 packed likewise: w1s[p, k, m] = 64*W1[m, k*128+p]  (fp8)
    w1_d = nc.dram_tensor("w1s", [128, NSUP, 2, MLP], fp8, kind="ExternalInput")
    # cwr: [W2.T/64 | wq32 (4 x 32-wide zero-padded blocks) | wv32]  (bf16)
    cwr_d = nc.dram_tensor("cwr", [MLP, MLP + 2 * P32 * NT], bf16,
                           kind="ExternalInput")
    # cb: bias columns [64*b1 | b2]  (f32)
    cb_d = nc.dram_tensor("cb", [MLP, 2], f32, kind="ExternalInput")
    ca_d = nc.dram_tensor("ca", [P32, 512], f32, kind="ExternalInput")
    st_d = nc.dram_tensor("stats", [P32, 2], f32, kind="ExternalOutput")

    with tile.TileContext(nc) as tc:
        with (
            tc.tile_pool(name="const", bufs=1) as const,
            tc.tile_pool(name="xp", bufs=(NSUP - 2) + NT + 4) as xp,
            tc.tile_pool(name="yp", bufs=4) as yp,
            tc.tile_pool(name="y2p", bufs=4) as y2p,
            tc.tile_pool(name="smallp", bufs=1) as smallp,
            tc.tile_pool(name="statsp", bufs=1) as statsp,
            tc.tile_pool(name="ps_y", bufs=4, space="PSUM") as ps_y,
            tc.tile_pool(name="ps_y2", bufs=2, space="PSUM") as ps_y2,
            tc.tile_pool(name="ps_q", bufs=1, space="PSUM") as ps_q,
            tc.tile_pool(name="ps_v", bufs=1, space="PSUM") as ps_v,
        ):
            # --- HAM warmup: zeroed fp8 tiles + dummy matmuls keep the PE
            # busy through one full 4096-cycle activity window during the
            # first x DMA, so the clock gate opens before real work lands.
            warm_w = const.tile([128, 2, MLP], fp8)
            nc.gpsimd.memset(warm_w[:], 0.0)
            warm_x = const.tile([128, 2, 512], fp8)
            nc.gpsimd.memset(warm_x[:], 0.0)
            warm_ps = ps_y2.tile([128, 512], f32, tag="y2", name="warm_ps")
            warm_mm = None
            for i in range(10):
                warm_mm = nc.tensor.matmul(warm_ps[:], warm_w[:], warm_x[:],
                                           start=True, stop=True,
                                           perf_mode=DR)

            # --- DMA plan.  Ring A (sync): x0,x2,x4,x6,xl0,xl2,stats.
            # Ring B (scalar): w1,x1,x3,x5,xl1,xl3.  Small consts on the
            # SWDGE ring.  Soft deps pin per-ring issue order; the lane
            # sems then deliver in consumption order.
            last_on = {}

            def ring_dma(ring_key, ring, **kw):
                dma = ring.dma_start(**kw)
                prev = last_on.get(ring_key)
                if prev is not None:
                    add_dep_helper(dma.ins, prev.ins, sync=False,
                                   reason="ring issue order")
                last_on[ring_key] = dma
                return dma

            w1_sb = const.tile([128, NSUP, 2, MLP], fp8)
            ring_dma('A', nc.sync, out=w1_sb[:], in_=w1_d[:])
            ca_sb = const.tile([P32, 512], f32)
            nc.gpsimd.dma_start(out=ca_sb[:], in_=ca_d[:])
            cb_sb = const.tile([MLP, 2], f32)
            nc.gpsimd.dma_start(out=cb_sb[:], in_=cb_d[:])
            cwr_sb = const.tile([MLP, MLP + 2 * P32 * NT], bf16)
            nc.gpsimd.dma_start(out=cwr_sb[:], in_=cwr_d[:])

            stats_sb = statsp.tile([P32, 2], f32)

            # One 512KB transfer per super-chunk, strictly alternating
            # rings: fine granularity keeps chunk latency ~1.5us so the PE
            # never idles past the HAM re-throttle window.  The first NSPL
            # supers are further split into token halves (0.25MB) so the
            # ramp delivers every ~0.8us; w1 leads ring A (ring B's first
            # issue sits behind the fixed ~1.5us ACT table load).
            NSPL = 4
            rings = [('B', nc.scalar), ('A', nc.sync)]
            x_tiles = []
            for g in range(NSUP - 2):
                rk, ring = rings[g % 2]
                if g < NSPL:
                    halves = []
                    for h in range(2):
                        xh = xp.tile([128, 2, TOK // 2], fp8, tag="x",
                                     name=f"x{g}h{h}")
                        ring_dma(rk, ring, out=xh[:],
                                 in_=xt_d[:, g, :,
                                          h * (TOK // 2):(h + 1) * (TOK // 2)])
                        halves.append(xh)
                    x_tiles.append(halves)
                else:
                    x_sb = xp.tile([128, 2, TOK], fp8, tag="x", name=f"x{g}")
                    ring_dma(rk, ring, out=x_sb[:], in_=xt_d[:, g])
                    x_tiles.append(x_sb)
            xl_tiles = []
            for t in range(NT):
                xl_sb = xp.tile([128, 2, 2, 512], fp8, tag="xl", name=f"xl{t}")
                rk, ring = rings[t % 2]
                ring_dma(rk, ring, out=xl_sb[:],
                         in_=xt_d[:, NSUP - 2:NSUP, :, t * 512:(t + 1) * 512])
                xl_tiles.append(xl_sb)

            # --- Layer 1: yT[t] (128, 512) += (64*W1)^T-chunk @ x-chunk,
            # DoubleRow accumulation over 16 super-chunks of K=256.
            psum_y = [ps_y.tile([128, 512], f32, tag="y", name=f"y_ps{t}")
                      for t in range(NT)]
            for g in range(NSUP - 2):
                for t in range(NT):
                    if g < NSPL:
                        half = x_tiles[g][t // 2]
                        rhs = half[:, :, (t % 2) * 512:(t % 2 + 1) * 512]
                    else:
                        rhs = x_tiles[g][:, :, t * 512:(t + 1) * 512]
                    mm = nc.tensor.matmul(
                        psum_y[t][:],
                        w1_sb[:, g],
                        rhs,
                        start=(g == 0), stop=False,
                        perf_mode=DR,
                    )
                    if g == 0 and t == 0 and warm_mm is not None:
                        add_dep_helper(mm.ins, warm_mm.ins, sync=False,
                                       reason="warmups before first mm")
            for t in range(NT):
                nc.tensor.matmul(psum_y[t][:], w1_sb[:, NSUP - 2],
                                 xl_tiles[t][:, 0],
                                 start=False, stop=False, perf_mode=DR)
                nc.tensor.matmul(psum_y[t][:], w1_sb[:, NSUP - 1],
                                 xl_tiles[t][:, 1],
                                 start=False, stop=True, perf_mode=DR)

            # cb/ca lane warmups (each engine observes the const lanes once
            # so steady-state instructions carry at most one new wait).
            warm_act = const.tile([MLP, 1], f32)
            nc.scalar.copy(out=warm_act[:], in_=cb_sb[:, 0:1])
            warm_dve = const.tile([P32, 1], f32)
            nc.vector.tensor_copy(out=warm_dve[:], in_=ca_sb[:, 0:1])
            warm_pe2 = ps_y2.tile([128, 512], f32, tag="y2", name="warm_pe2")
            nc.tensor.matmul(warm_pe2[0:NH, 0:NH], cwr_sb[:, 0:NH],
                             cwr_sb[:, 0:NH], start=True, stop=True)

            # --- MLP tail + head projections per token tile (bf16).
            q32_ps = ps_q.tile([P32, 512], f32, tag="q", name="q32_ps")
            v32_ps = ps_v.tile([P32, 512], f32, tag="v", name="v32_ps")
            for t in range(NT):
                y_sb = yp.tile([128, 512], bf16, tag="ysb", name=f"y_sb{t}")
                # relu on DVE (add+max) keeps ACT free for relu2/exp; the
                # 64x scale rides along and is cancelled by W2/64 in cwr.
                nc.vector.tensor_scalar(out=y_sb[:], in0=psum_y[t][:],
                                        scalar1=cb_sb[:, 0:1],
                                        scalar2=0.0, op0=OP.add, op1=OP.max)
                y2_ps = ps_y2.tile([128, 512], f32, tag="y2", name=f"y2_ps{t}")
                nc.tensor.matmul(y2_ps[:], cwr_sb[:, 0:MLP], y_sb[:],
                                 start=True, stop=True)
                y2_sb = y2p.tile([128, 512], bf16, tag="y2sb", name=f"y2_sb{t}")
                nc.scalar.activation(out=y2_sb[:], in_=y2_ps[:], func=AF.Relu,
                                     bias=cb_sb[:, 1:2], scale=1.0)
                # Head projections: the (128, 32) weight block for tile t is
                # zero outside rows 8t..8t+8, so accumulating all 4 tiles into
                # one (32, 512) bank packs q/v as (tile, head) x seq lanes.
                nc.tensor.matmul(q32_ps[:],
                                 cwr_sb[:, CQ + P32 * t:CQ + P32 * (t + 1)],
                                 y2_sb[:], start=(t == 0), stop=(t == NT - 1))
                nc.tensor.matmul(v32_ps[:],
                                 cwr_sb[:, CV + P32 * t:CV + P32 * (t + 1)],
                                 y2_sb[:], start=(t == 0), stop=(t == NT - 1))

            # --- Softmax stats over the packed (32, 512) lanes.
            # ca already contains ramp + mask - K, so l' = q + ca is the
            # shifted logit; no max-reduce needed on device.
            l_sb = smallp.tile([P32, 512], f32, tag="l", name="l_sb")
            nc.vector.tensor_add(out=l_sb[:], in0=q32_ps[:],
                                 in1=ca_sb[:])
            e_sb = smallp.tile([P32, 512], f32, tag="e", name="e_sb")
            # e = exp(l'); stats[:, 0] = Z = sum e
            nc.scalar.activation(out=e_sb[:], in_=l_sb[:], func=AF.Exp,
                                 bias=0.0, scale=1.0,
                                 accum_out=stats_sb[:, 0:1])
            ev_sb = smallp.tile([P32, 512], f32, tag="ev", name="ev_sb")
            nc.vector.tensor_mul(out=ev_sb[:], in0=e_sb[:], in1=v32_ps[:])
            # stats[:, 1] = W' = sum e*v_raw   (bv folded in on host)
            nc.vector.tensor_reduce(out=stats_sb[:, 1:2], in_=ev_sb[:],
                                    axis=AX.X, op=OP.add)

            ring_dma('A', nc.sync, out=st_d[:], in_=stats_sb[:])

    nc.finalize()
    return nc


def get_nc():
    if "nc" not in _cache:
        _cache["nc"] = _build_nc()
    return _cache["nc"]


def make_core_inputs(x, mask, W1, b1, W2, b2, Wq, Wv, bv, pos_w, bias):
    """Host-side shard + transpose + fp8 quantization.

    Returns (in_maps, K) where K[c, t, h] is the logit shift baked into
    core c's `ca` (the host-known max of ramp+mask over the shard)."""
    import ml_dtypes
    fp8 = ml_dtypes.float8_e4m3
    bf16 = ml_dtypes.bfloat16

    # w1s[p, k, m] = 64*W1[m, k*128+p], fp8
    w1s = np.ascontiguousarray(
        (W1 * W1_SCALE).reshape(MLP, KCH, 128).transpose(2, 1, 0)
    ).astype(fp8).reshape(128, NSUP, 2, MLP)

    cwr = np.zeros((MLP, MLP + 2 * P32 * NT), dtype=np.float32)
    cwr[:, 0:MLP] = W2.T / W1_SCALE
    # zero-padded per-tile head blocks: block t covers psum rows 8t..8t+8
    for t in range(NT):
        cwr[:, MLP + P32 * t + NH * t:MLP + P32 * t + NH * (t + 1)] = Wq.T
        base_v = MLP + P32 * NT
        cwr[:, base_v + P32 * t + NH * t:base_v + P32 * t + NH * (t + 1)] = \
            Wv.T
    cwr = cwr.astype(bf16)
    cb = np.stack([b1 * W1_SCALE, b2], axis=1).astype(np.float32)  # (MLP, 2)
    pos = np.arange(S, dtype=np.float32)
    maskadd = np.where(mask == 0, np.float32(-1e9), np.float32(0.0))  # (B,S)

    in_maps = []
    K_all = np.empty((NCORES, NT, NH), dtype=np.float64)
    for c in range(NCORES):
        sl = slice(c * S_SHARD, (c + 1) * S_SHARD)
        # xt[p, k, n] = x^T[k*128+p, n] for this shard's 2048 tokens
        xt = np.ascontiguousarray(
            x[:, sl, :].astype(fp8).transpose(2, 0, 1).reshape(H, TOK)
            .reshape(KCH, 128, TOK).transpose(1, 0, 2)
        ).reshape(128, NSUP, 2, TOK)
        add_ths = (pos_w.astype(np.float64)[None, :, None]
                   * pos[sl].astype(np.float64)[None, None, :]
                   + maskadd[:, None, sl])           # (B=NT, NH, 512)
        # K: host-known shift; floor keeps exp() sane when fully masked
        K = np.maximum(add_ths.max(axis=2), -60.0)   # (NT, NH)
        K_all[c] = K
        ca = (add_ths - K[:, :, None]).astype(np.float32).reshape(P32, 512)
        in_maps.append({"xt": xt, "w1s": w1s, "cwr": cwr, "cb": cb,
                        "ca": ca})
    return in_maps, K_all


def merge_stats(stats_all, K_all, bv, bias):
    """stats_all: (NCORES, 32, 2) = [Z, W'] per (batch tile, head) lane
    under shift K_all[c, t, h]; W = W' + bv*Z -> (B, 1) output."""
    st = np.asarray(stats_all, dtype=np.float64).reshape(NCORES, NT, NH, 2)
    m = K_all                # (C, B, NH): logits were shifted by -K
    Z = st[..., 0]
    W = st[..., 1] + bv.astype(np.float64)[None, None, :] * Z
    M = m.max(axis=0)        # (B, NH)
    alpha = np.exp(m - M[None])
    Zg = (alpha * Z).sum(axis=0)
    Wg = (alpha * W).sum(axis=0)
    out = (Wg / Zg).sum(axis=1)          # (B,)
    return (out[:, None] + np.float64(bias.reshape(1)[0])).astype(np.float32)


def kernel(x, mask, W1, b1, W2, b2, Wq, Wv, bv, pos_w, bias, _trace=False):
    from concourse.bass_utils import run_bass_kernel_spmd

    x = np.asarray(x, dtype=np.float32)
    args = [np.asarray(a) for a in (W1, b1, W2, b2, Wq, Wv, bv, pos_w, bias)]
    in_maps, K_all = make_core_inputs(x, np.asarray(mask), *args)
    nc = get_nc()
    res = run_bass_kernel_spmd(nc, in_maps, core_ids=list(range(NCORES)),
                               trace=_trace)
    stats_all = np.stack([r["stats"] for r in res.results])  # (C, 32, 2)
    out = merge_stats(stats_all, K_all, args[6], args[8])
    if _trace:
        kernel.last_result = res
    return out


# revision 26
# speedup vs baseline: 1.1683x; 1.1671x over previous
"""Trainium2 Bass kernel for nn_AttentionProbe_80891414053184.

Math (reference):
    y  = relu(x @ W1.T + b1)            # (B,S,H) -> (B,S,128)
    y2 = relu(y @ W2.T + b2)            # (B,S,128)
    l  = y2 @ Wq.T + pos*pos_w  (+mask) # (B,S,8) logits
    p  = softmax(l, axis=S)
    v  = y2 @ Wv.T + bv
    out[b] = sum_{s,h} p*v + bias       # (B,1)

Strategy: sequence-parallel over 8 cores (512 positions x 4 batches = 2048
tokens per core).  Each core streams its x-shard, runs the MLP + head
projections on-chip, and emits per-(batch, head) partial softmax stats
(Z=sum exp(l-K), W'=sum exp(l-K)*v_raw) for a HOST-CHOSEN shift K (the
max of the ALiBi ramp + mask term over the shard, known without looking
at the data).  The host merges the 8 partial stats with the standard
online-softmax combine (m=K per core) and produces the (4,1) output.

Perf decisions (from HW traces of earlier versions):
 - x and W1 travel as fp8 e4m3 (W1 pre-scaled by 64 so its sigma~1/64
   values leave the fp8 denormal range; 1/64 is folded into W2, exact by
   relu's positive homogeneity).  Halves the dominant HBM stream vs bf16.
 - Layer-1 matmuls run perf_mode=DoubleRow: K=256 per instruction.
 - x streams via BOTH HWDGE rings (nc.sync + nc.scalar) in 512KB
   super-chunk transfers, soft-dep-chained so per-ring delivery order ==
   PE consumption order; the first supers are split in token halves so
   the ramp delivers every ~0.8us.  w1 leads ring A (ring B's first issue
   sits behind the fixed ~1.5us ACT exp-table load).
 - W2/Wq/Wv and the MLP tail activations run in bf16; measured
   end-to-end rel-err ~6e-3 against a 2e-2 gate.
 - The whole fp8 x-shard (64KB/partition) stays resident in SBUF.
 - The last super-chunk pair is DMA'd per token tile so tile t's MLP tail
   overlaps tile t+1's final transfer.
 - q and v projections land in ONE psum bank per column half (q rows
   0:32, v rows 32:64, zero-padded per-tile weight blocks), and the last
   tile's tail + softmax-stats chain are split into column halves so the
   post-stream critical path is [32,256]-sized ops pipelined across
   DVE/ACT/PE instead of a serial [32,512] chain.
 - Dummy matmuls on zeroed SBUF bridge the PE through the DMA ramp and
   1-2 filler matmuls per super keep the HAM clock gate at full rate
   through the stream (a few idle microseconds re-throttle the PE to
   1.2GHz for 10+us).
 - stats[:,0:2]=Z halves, stats[:,2:4]=W' halves; Z/W' summing, the bv
   fold (W=W'+bv*Z) and the cross-core merge happen on host.
"""

import numpy as np

# Problem dims (hardcoded per harness contract).
B, S, H = 4, 4096, 4096
MLP, NH = 128, 8
NCORES = 8
S_SHARD = S // NCORES        # 512 seq positions per core
TOK = B * S_SHARD            # 2048 tokens per core
NT = TOK // 512              # 4 token tiles of 512 (= one batch each)
KCH = H // 128               # 32 contraction chunks of 128
NSUP = KCH // 2              # 16 DoubleRow super-chunks of 256
P32 = NT * NH                # 32 packed (tile, head) lanes
W1_SCALE = 64.0              # lifts W1 (sigma 1/64) out of fp8 denormals

_cache = {}


def _build_nc():
    import concourse.mybir as mybir
    import concourse.tile as tile
    from concourse import bacc
    from concourse.tile import add_dep_helper

    f32 = mybir.dt.float32
    bf16 = mybir.dt.bfloat16
    fp8 = mybir.dt.float8e4
    DR = mybir.MatmulPerfMode.DoubleRow

    AF = mybir.ActivationFunctionType
    OP = mybir.AluOpType
    CQV = MLP                   # combined q|v 64-wide blocks start (cwr)

    nc = bacc.Bacc()
    # x, packed on host: xt[p, k, n] = x^T[k*128+p, n]  (fp8)
    xt_d = nc.dram_tensor("xt", [128, NSUP, 2, TOK], fp8, kind="ExternalInput")
    # W1*64 packed likewise: w1s[p, k, m] = 64*W1[m, k*128+p]  (fp8)
    w1_d = nc.dram_tensor("w1s", [128, NSUP, 2, MLP], fp8, kind="ExternalInput")
    # cwr: [W2.T/64 | per-tile 64-wide q|v blocks]  (bf16)
    cwr_d = nc.dram_tensor("cwr", [MLP, MLP + 64 * NT], bf16,
                           kind="ExternalInput")
    # cb: bias columns [64*b1 | b2]  (f32)
    cb_d = nc.dram_tensor("cb", [MLP, 2], f32, kind="ExternalInput")
    ca_d = nc.dram_tensor("ca", [P32, 512], f32, kind="ExternalInput")
    st_d = nc.dram_tensor("stats", [P32, 4], f32, kind="ExternalOutput")

    with tile.TileContext(nc) as tc:
        with (
            tc.tile_pool(name="const", bufs=1) as const,
            tc.tile_pool(name="xp", bufs=(NSUP - 2) + NT + 4) as xp,
            tc.tile_pool(name="yp", bufs=4) as yp,
            tc.tile_pool(name="y2p", bufs=4) as y2p,
            tc.tile_pool(name="smallp", bufs=1) as smallp,
            tc.tile_pool(name="statsp", bufs=1) as statsp,
            tc.tile_pool(name="ps_y", bufs=4, space="PSUM") as ps_y,
            tc.tile_pool(name="ps_y2", bufs=1, space="PSUM") as ps_y2,
            tc.tile_pool(name="ps_qv", bufs=1, space="PSUM") as ps_qv,
        ):
            # qv banks: [q lanes 0:32 | v lanes 32:64] x 512, one bank per
            # column half (the half chains read one bank while the PE still
            # writes the other).  Warm/filler matmuls park in qv0's bank.
            qv_ps = [ps_qv.tile([64, 512], f32, tag=f"qv{h}", name=f"qv{h}")
                     for h in range(2)]

            # --- HAM warmup: zeroed fp8 tiles + dummy matmuls keep the PE
            # busy through the DMA ramp so the clock gate opens before real
            # work lands.
            warm_w = const.tile([128, 2, 64], fp8)
            nc.gpsimd.memset(warm_w[:], 0.0)
            warm_x = const.tile([128, 2, 512], fp8)
            nc.gpsimd.memset(warm_x[:], 0.0)
            warm_mm = None
            for i in range(10):
                warm_mm = nc.tensor.matmul(qv_ps[0][:], warm_w[:], warm_x[:],
                                           start=True, stop=True,
                                           perf_mode=DR)

            # --- DMA plan.  Ring A (sync) leads with w1; x super-chunks
            # alternate rings B,A,B,... in consumption order (soft deps pin
            # per-ring issue order).  Small consts ride the SWDGE ring.
            last_on = {}

            def ring_dma(ring_key, ring, **kw):
                dma = ring.dma_start(**kw)
                prev = last_on.get(ring_key)
                if prev is not None:
                    add_dep_helper(dma.ins, prev.ins, sync=False,
                                   reason="ring issue order")
                last_on[ring_key] = dma
                return dma

            w1_sb = const.tile([128, NSUP, 2, MLP], fp8)
            ring_dma('A', nc.sync, out=w1_sb[:], in_=w1_d[:])
            ca_sb = const.tile([P32, 512], f32)
            nc.gpsimd.dma_start(out=ca_sb[:], in_=ca_d[:])
            cb_sb = const.tile([MLP, 2], f32)
            nc.gpsimd.dma_start(out=cb_sb[:], in_=cb_d[:])
            cwr_sb = const.tile([MLP, MLP + 64 * NT], bf16)
            nc.gpsimd.dma_start(out=cwr_sb[:], in_=cwr_d[:])

            stats_sb = statsp.tile([P32, 4], f32)

            NSPL = 4
            rings = [('B', nc.scalar), ('A', nc.sync)]
            x_tiles = []
            for g in range(NSUP - 2):
                rk, ring = rings[g % 2]
                if g < NSPL:
                    halves = []
                    for h in range(2):
                        xh = xp.tile([128, 2, TOK // 2], fp8, tag="x",
                                     name=f"x{g}h{h}")
                        ring_dma(rk, ring, out=xh[:],
                                 in_=xt_d[:, g, :,
                                          h * (TOK // 2):(h + 1) * (TOK // 2)])
                        halves.append(xh)
                    x_tiles.append(halves)
                else:
                    x_sb = xp.tile([128, 2, TOK], fp8, tag="x", name=f"x{g}")
                    ring_dma(rk, ring, out=x_sb[:], in_=xt_d[:, g])
                    x_tiles.append(x_sb)
            xl_tiles = []
            for t in range(NT):
                xl_sb = xp.tile([128, 2, 2, 512], fp8, tag="xl", name=f"xl{t}")
                rk, ring = rings[t % 2]
                ring_dma(rk, ring, out=xl_sb[:],
                         in_=xt_d[:, NSUP - 2:NSUP, :, t * 512:(t + 1) * 512])
                xl_tiles.append(xl_sb)

            # --- Layer 1: yT[t] (128, 512) += (64*W1)^T-chunk @ x-chunk,
            # DoubleRow accumulation over 16 super-chunks of K=256.
            psum_y = [ps_y.tile([128, 512], f32, tag="y", name=f"y_ps{t}")
                      for t in range(NT)]
            for g in range(NSUP - 2):
                for t in range(NT):
                    if g < NSPL:
                        half = x_tiles[g][t // 2]
                        rhs = half[:, :, (t % 2) * 512:(t % 2 + 1) * 512]
                    else:
                        rhs = x_tiles[g][:, :, t * 512:(t + 1) * 512]
                    mm = nc.tensor.matmul(
                        psum_y[t][:],
                        w1_sb[:, g],
                        rhs,
                        start=(g == 0), stop=False,
                        perf_mode=DR,
                    )
                    if g == 0 and t == 0 and warm_mm is not None:
                        add_dep_helper(mm.ins, warm_mm.ins, sync=False,
                                       reason="warmups before first mm")
                # Filler matmuls on resident zeros: the PE is ~40% idle
                # while tracking the DMA stream, and a few contiguous idle
                # microseconds re-throttle the HAM clock gate (half-rate
                # matmuls for 10+us).  Always-ready fillers per super keep
                # every 4096-cycle activity window busy.
                if g < 12:
                    for _ in range(2):
                        nc.tensor.matmul(qv_ps[0][:], warm_w[:], warm_x[:],
                                         start=True, stop=True, perf_mode=DR)
            for t in range(NT):
                nc.tensor.matmul(psum_y[t][:], w1_sb[:, NSUP - 2],
                                 xl_tiles[t][:, 0],
                                 start=False, stop=False, perf_mode=DR)
                nc.tensor.matmul(psum_y[t][:], w1_sb[:, NSUP - 1],
                                 xl_tiles[t][:, 1],
                                 start=False, stop=True, perf_mode=DR)

            # cb/ca lane warmups (each engine observes the const lanes once
            # so steady-state instructions carry at most one new wait).
            warm_act = const.tile([MLP, 1], f32)
            nc.scalar.copy(out=warm_act[:], in_=cb_sb[:, 0:1])
            warm_dve = const.tile([P32, 1], f32)
            nc.vector.tensor_copy(out=warm_dve[:], in_=ca_sb[:, 0:1])
            nc.tensor.matmul(qv_ps[1][0:NH, 0:NH], cwr_sb[:, 0:NH],
                             cwr_sb[:, 0:NH], start=True, stop=True)

            # --- MLP tail + head projections (bf16).  Tiles 0..NT-2 run
            # full-width; the last tile is split into column halves so the
            # stats chain starts on half 0 while half 1 still computes.
            # qv accumulation flags: within each qv bank the FIRST matmul
            # (tile 0) uses start=True, the last (tile NT-1) stop=True.
            y2_banks = [ps_y2.tile([128, 512], f32, tag=f"y2b{i}",
                                   name=f"y2b{i}") for i in range(2)]

            def qv_block(t):
                return cwr_sb[:, CQV + 64 * t:CQV + 64 * (t + 1)]

            for t in range(NT - 1):
                y_sb = yp.tile([128, 512], bf16, tag="ysb", name=f"y_sb{t}")
                # relu on DVE (add+max) keeps ACT free for relu2/exp; the
                # 64x scale rides along and is cancelled by W2/64 in cwr.
                nc.vector.tensor_scalar(out=y_sb[:], in0=psum_y[t][:],
                                        scalar1=cb_sb[:, 0:1],
                                        scalar2=0.0, op0=OP.add, op1=OP.max)
                y2_ps = y2_banks[t % 2]
                nc.tensor.matmul(y2_ps[:], cwr_sb[:, 0:MLP], y_sb[:],
                                 start=True, stop=True)
                y2_sb = y2p.tile([128, 512], bf16, tag="y2sb", name=f"y2_sb{t}")
                nc.scalar.activation(out=y2_sb[:], in_=y2_ps[:], func=AF.Relu,
                                     bias=cb_sb[:, 1:2], scale=1.0)
                for h in range(2):
                    nc.tensor.matmul(
                        qv_ps[h][:, 0:256], qv_block(t),
                        y2_sb[:, 256 * h:256 * (h + 1)],
                        start=(t == 0), stop=False,
                        skip_group_check=True)

            # Last tile, split by column halves.
            t = NT - 1
            y_sb3 = yp.tile([128, 512], bf16, tag="ysb", name="y_sb3")
            y2_sb3 = y2p.tile([128, 512], bf16, tag="y2sb", name="y2_sb3")
            l_sb = smallp.tile([P32, 512], f32, tag="l", name="l_sb")
            e_sb = smallp.tile([P32, 512], f32, tag="e", name="e_sb")
            ev_sb = smallp.tile([P32, 512], f32, tag="ev", name="ev_sb")
            for h in range(2):
                cols = slice(256 * h, 256 * (h + 1))
                nc.vector.tensor_scalar(out=y_sb3[:, cols],
                                        in0=psum_y[t][:, cols],
                                        scalar1=cb_sb[:, 0:1],
                                        scalar2=0.0, op0=OP.add, op1=OP.max)
                y2_ps = y2_banks[(t + h) % 2]
                nc.tensor.matmul(y2_ps[:, 0:256], cwr_sb[:, 0:MLP],
                                 y_sb3[:, cols], start=True, stop=True)
                nc.scalar.activation(out=y2_sb3[:, cols],
                                     in_=y2_ps[:, 0:256], func=AF.Relu,
                                     bias=cb_sb[:, 1:2], scale=1.0)
                nc.tensor.matmul(qv_ps[h][:, 0:256], qv_block(t),
                                 y2_sb3[:, cols],
                                 start=False, stop=True,
                                 skip_group_check=True)

                # --- Softmax stats for this half: ca already contains
                # ramp + mask - K, so l' = q + ca is the shifted logit.
                nc.vector.tensor_add(out=l_sb[:, cols],
                                     in0=qv_ps[h][0:P32, 0:256],
                                     in1=ca_sb[:, cols])
                # e = exp(l'); stats[:, h] = Z_h = sum e
                nc.scalar.activation(out=e_sb[:, cols], in_=l_sb[:, cols],
                                     func=AF.Exp, bias=0.0, scale=1.0,
                                     accum_out=stats_sb[:, h:h + 1])
                # stats[:, 2+h] = W'_h = sum e*v_raw in one fused DVE op
                # (bv folded in on host)
                nc.vector.scalar_tensor_tensor(
                    out=ev_sb[:, cols], in0=e_sb[:, cols], scalar=0.0,
                    in1=qv_ps[h][P32:2 * P32, 0:256],
                    op0=OP.add, op1=OP.mult,
                    accum_out=stats_sb[:, 2 + h:3 + h])

            ring_dma('A', nc.sync, out=st_d[:], in_=stats_sb[:])

    nc.finalize()
    return nc


def get_nc():
    if "nc" not in _cache:
        _cache["nc"] = _build_nc()
    return _cache["nc"]


def make_core_inputs(x, mask, W1, b1, W2, b2, Wq, Wv, bv, pos_w, bias):
    """Host-side shard + transpose + fp8 quantization.

    Returns (in_maps, K) where K[c, t, h] is the logit shift baked into
    core c's `ca` (the host-known max of ramp+mask over the shard)."""
    import ml_dtypes
    fp8 = ml_dtypes.float8_e4m3
    bf16 = ml_dtypes.bfloat16

    # w1s[p, k, m] = 64*W1[m, k*128+p], fp8
    w1s = np.ascontiguousarray(
        (W1 * W1_SCALE).reshape(MLP, KCH, 128).transpose(2, 1, 0)
    ).astype(fp8).reshape(128, NSUP, 2, MLP)

    # cwr: [W2.T/64 | per-tile 64-wide combined q|v blocks]; block t rows
    # 8t..8t+8 carry Wq (cols 0:32) and Wv (cols 32:64), zero elsewhere,
    # so all tiles accumulate into one [q;v] psum bank per column half.
    cwr = np.zeros((MLP, MLP + 64 * NT), dtype=np.float32)
    cwr[:, 0:MLP] = W2.T / W1_SCALE
    for t in range(NT):
        base = MLP + 64 * t
        cwr[:, base + NH * t:base + NH * (t + 1)] = Wq.T
        cwr[:, base + 32 + NH * t:base + 32 + NH * (t + 1)] = Wv.T
    cwr = cwr.astype(bf16)
    cb = np.stack([b1 * W1_SCALE, b2], axis=1).astype(np.float32)  # (MLP, 2)
    pos = np.arange(S, dtype=np.float32)
    maskadd = np.where(mask == 0, np.float32(-1e9), np.float32(0.0))  # (B,S)

    in_maps = []
    K_all = np.empty((NCORES, NT, NH), dtype=np.float64)
    for c in range(NCORES):
        sl = slice(c * S_SHARD, (c + 1) * S_SHARD)
        # xt[p, k, n] = x^T[k*128+p, n] for this shard's 2048 tokens
        xt = np.ascontiguousarray(
            x[:, sl, :].astype(fp8).transpose(2, 0, 1).reshape(H, TOK)
            .reshape(KCH, 128, TOK).transpose(1, 0, 2)
        ).reshape(128, NSUP, 2, TOK)
        add_ths = (pos_w.astype(np.float64)[None, :, None]
                   * pos[sl].astype(np.float64)[None, None, :]
                   + maskadd[:, None, sl])           # (B=NT, NH, 512)
        # K: host-known shift; floor keeps exp() sane when fully masked
        K = np.maximum(add_ths.max(axis=2), -60.0)   # (NT, NH)
        K_all[c] = K
        ca = (add_ths - K[:, :, None]).astype(np.float32).reshape(P32, 512)
        in_maps.append({"xt": xt, "w1s": w1s, "cwr": cwr, "cb": cb,
                        "ca": ca})
    return in_maps, K_all


def merge_stats(stats_all, K_all, bv, bias):
    """stats_all: (NCORES, 32, 4) = [Z_h0, Z_h1, W'_h0, W'_h1] per
    (batch tile, head) lane under shift K_all[c, t, h]; W = W' + bv*Z
    -> (B, 1) output."""
    st = np.asarray(stats_all, dtype=np.float64).reshape(NCORES, NT, NH, 4)
    m = K_all                # (C, B, NH): logits were shifted by -K
    Z = st[..., 0] + st[..., 1]
    W = st[..., 2] + st[..., 3] + bv.astype(np.float64)[None, None, :] * Z
    M = m.max(axis=0)        # (B, NH)
    alpha = np.exp(m - M[None])
    Zg = (alpha * Z).sum(axis=0)
    Wg = (alpha * W).sum(axis=0)
    out = (Wg / Zg).sum(axis=1)          # (B,)
    return (out[:, None] + np.float64(bias.reshape(1)[0])).astype(np.float32)


def kernel(x, mask, W1, b1, W2, b2, Wq, Wv, bv, pos_w, bias, _trace=False):
    from concourse.bass_utils import run_bass_kernel_spmd

    x = np.asarray(x, dtype=np.float32)
    args = [np.asarray(a) for a in (W1, b1, W2, b2, Wq, Wv, bv, pos_w, bias)]
    in_maps, K_all = make_core_inputs(x, np.asarray(mask), *args)
    nc = get_nc()
    res = run_bass_kernel_spmd(nc, in_maps, core_ids=list(range(NCORES)),
                               trace=_trace)
    stats_all = np.stack([r["stats"] for r in res.results])  # (C, 32, 4)
    out = merge_stats(stats_all, K_all, args[6], args[8])
    if _trace:
        kernel.last_result = res
    return out
